# revision 1
# baseline (speedup 1.0000x reference)
"""Trainium2 Bass kernel: causal multi-head attention with interleaved RoPE.

Problem shapes (hardcoded): x [2, 2048, 1024], 16 heads of dk=64.
Sharding: 8 cores = 2 batches x 4 head-groups (4 heads each). Each core
computes its head-slice Q/K/V projections, RoPE, causal attention, and a
partial output through its Wo row-slice; the host sums the 4 partials per
batch and adds bo.

RoPE trick: attention scores are invariant to any permutation of the dk
axis applied to both Q and K, so the Wq/Wk columns are permuted on the host
into a "quadrant half-split" layout where each rotation pair partner sits
exactly 16 partitions away inside the same 32-partition quadrant. The DVE
stream_shuffle (a per-quadrant 32-way permute) then produces the swapped
operand, and RoPE becomes: rot = q * cosT + shuffle(q) * sinT with
host-precomputed tables (sinT carries the sign).
"""

import os
from contextlib import ExitStack

import numpy as np

import concourse.bass as bass
import concourse.mybir as mybir
import concourse.tile as tile

B, S, D, H = 2, 2048, 1024, 16
DK = D // H  # 64
HG = 4  # heads per core
NCOLS = HG * DK  # 256 columns of the projection per core
THETA = 10000.0
SCALE = 1.0 / float(np.sqrt(DK))
N_CORES = 8

F32 = mybir.dt.float32
F32R = mybir.dt.float32r

# matmul operand dtype: float32r (= TF32, 10-bit mantissa) streams 1 col/cycle
# on the PE vs 4 for float32. Operands must be *rounded* to TF32: DMA-fed
# tensors are pre-rounded on the host and declared float32r; on-chip operand
# producers write float32r directly. Numerics validated in test.py.
USE_F32R = os.environ.get("KERNEL_F32", "0") != "1"
MMDT = F32R if USE_F32R else F32


def round_tf32(a):
    """Round fp32 array to TF32 (RNE to 10-bit mantissa)."""
    if not USE_F32R:
        return np.ascontiguousarray(a, dtype=np.float32)
    u = np.ascontiguousarray(a, dtype=np.float32).view(np.uint32).copy()
    u += 0x0FFF + ((u >> 13) & 1)
    u &= np.uint32(0xFFFFE000)
    return u.view(np.float32)


# ---------------------------------------------------------------------------
# host-side prep
# ---------------------------------------------------------------------------

def _rope_perm():
    """Within-head column permutation pi: new row r -> original dk index."""
    perm = np.empty(DK, dtype=np.int64)
    for r in range(DK):
        q, m = divmod(r, 32)
        if m < 16:
            perm[r] = 2 * (16 * q + m)
        else:
            perm[r] = 2 * (16 * q + m - 16) + 1
    return perm


_PERM = _rope_perm()
SHUF_MASK = list(range(16, 32)) + list(range(16))  # swap 16-halves per quadrant


def _rope_tables(pos):
    """cosT/sinT [128, S] fp32 for the permuted layout. pos: [S] int."""
    inv_freq = (np.float32(THETA) ** (-(np.arange(0, DK, 2, dtype=np.float32) / np.float32(DK))))  # [32]
    ang = pos.astype(np.float32)[:, None] * inv_freq[None, :]  # [S, 32]
    cos = np.cos(ang)  # [S, 32]
    sin = np.sin(ang)
    cosT = np.empty((128, S), dtype=np.float32)
    sinT = np.empty((128, S), dtype=np.float32)
    for p in range(128):
        r = p % DK
        q, m = divmod(r, 32)
        if m < 16:
            i = 16 * q + m
            sgn = -1.0
        else:
            i = 16 * q + m - 16
            sgn = 1.0
        cosT[p] = cos[:, i]
        sinT[p] = np.float32(sgn) * sin[:, i]
    return cosT, sinT


def make_core_inputs(x, token_position, Wq, bq, Wk, bk, Wv, bv, Wo, bo):
    """Build the 8 per-core input maps."""
    x = np.asarray(x, dtype=np.float32)
    token_position = np.asarray(token_position)
    Wq, Wk, Wv, Wo = (np.asarray(w, dtype=np.float32) for w in (Wq, Wk, Wv, Wo))
    bq, bk, bv = (np.asarray(b_, dtype=np.float32) for b_ in (bq, bk, bv))

    in_maps = []
    tables = {}
    for c in range(N_CORES):
        b, hg = divmod(c, HG)
        heads = range(HG * hg, HG * hg + HG)
        # permuted q/k column indices for this core's heads
        cols_qk = np.concatenate([DK * h + _PERM for h in heads])
        cols_v = np.arange(NCOLS * hg, NCOLS * hg + NCOLS)
        if b not in tables:
            tables[b] = _rope_tables(np.asarray(token_position[b]))
        cosT, sinT = tables[b]
        wo_rows = Wo[cols_v, :]  # [256, 1024]
        in_maps.append({
            "xT": round_tf32(x[b].T),                               # [1024, 2048]
            "wq": round_tf32(Wq[:, cols_qk]),                       # [1024, 256]
            "wk": round_tf32(Wk[:, cols_qk]),
            "wv": round_tf32(Wv[:, cols_v]),
            "wo": round_tf32(wo_rows.reshape(HG, DK, D).transpose(1, 0, 2)),  # [64, 4, 1024]
            "bq": round_tf32(bq[cols_qk][None, :]),                 # [1, 256]
            "bk": round_tf32(bk[cols_qk][None, :]),
            "bv": round_tf32(bv[cols_v][None, :]),
            "ones_row": round_tf32(np.ones((1, 512), np.float32)),
            "onesc": round_tf32(np.ones((128, 64), np.float32)),
            "cosT": cosT,
            "sinT": sinT,
        })
    return in_maps


# ---------------------------------------------------------------------------
# device program
# ---------------------------------------------------------------------------

def build_program(with_bias=False):
    from concourse import bacc, library_config
    nc = bacc.Bacc("TRN2", debug=False)

    xT = nc.declare_dram_parameter("xT", [D, S], MMDT, isOutput=False).ap()
    wq = nc.declare_dram_parameter("wq", [D, NCOLS], MMDT, isOutput=False).ap()
    wk = nc.declare_dram_parameter("wk", [D, NCOLS], MMDT, isOutput=False).ap()
    wv = nc.declare_dram_parameter("wv", [D, NCOLS], MMDT, isOutput=False).ap()
    wo = nc.declare_dram_parameter("wo", [DK, HG, D], MMDT, isOutput=False).ap()
    bq = nc.declare_dram_parameter("bq", [1, NCOLS], MMDT, isOutput=False).ap()
    bk = nc.declare_dram_parameter("bk", [1, NCOLS], MMDT, isOutput=False).ap()
    bv = nc.declare_dram_parameter("bv", [1, NCOLS], MMDT, isOutput=False).ap()
    ones_row_d = nc.declare_dram_parameter("ones_row", [1, 512], MMDT, isOutput=False).ap()
    onesc_d = nc.declare_dram_parameter("onesc", [128, DK], MMDT, isOutput=False).ap()
    cosT = nc.declare_dram_parameter("cosT", [128, S], F32, isOutput=False).ap()
    sinT = nc.declare_dram_parameter("sinT", [128, S], F32, isOutput=False).ap()
    out = nc.declare_dram_parameter("out", [S, D], F32, isOutput=True).ap()

    SB = 512            # sq block width
    NSB = S // SB       # 4
    NST = S // 128      # 16 key tiles / V tiles
    NDC = D // 128      # 8 contraction chunks
    GW = 2              # key tiles per score-psum group

    with tile.TileContext(nc) as tc, ExitStack() as ctx:
        nc.gpsimd.load_library(library_config.proxy)
        const = ctx.enter_context(tc.tile_pool(name="const", bufs=1))
        sbig = ctx.enter_context(tc.tile_pool(name="sbig", bufs=1))
        xts = ctx.enter_context(tc.tile_pool(name="xts", bufs=4))
        rtmp = ctx.enter_context(tc.tile_pool(name="rtmp", bufs=2))
        epool = ctx.enter_context(tc.tile_pool(name="epool", bufs=3))
        npool = ctx.enter_context(tc.tile_pool(name="npool", bufs=3))
        opool = ctx.enter_context(tc.tile_pool(name="opool", bufs=2))

        # --- constants / weights resident in SBUF (per-dc tiles: finer deps,
        # so the first projection matmuls start after ~128KB of DMA)
        wq_sb = [const.tile([128, NCOLS], MMDT, tag=f"wq{dc}", name=f"wq{dc}")
                 for dc in range(NDC)]
        wk_sb = [const.tile([128, NCOLS], MMDT, tag=f"wk{dc}", name=f"wk{dc}")
                 for dc in range(NDC)]
        wv_sb = [const.tile([128, NCOLS], MMDT, tag=f"wv{dc}", name=f"wv{dc}")
                 for dc in range(NDC)]
        for dc in range(NDC):
            nc.sync.dma_start(wq_sb[dc][:], wq[128 * dc:128 * dc + 128, :])
            nc.sync.dma_start(wk_sb[dc][:], wk[128 * dc:128 * dc + 128, :])
        cos_sb = const.tile([128, S], F32, tag="cos")
        sin_sb = const.tile([128, S], F32, tag="sin")
        nc.sync.dma_start(cos_sb[:], cosT)
        nc.sync.dma_start(sin_sb[:], sinT)
        for dc in range(NDC):
            nc.sync.dma_start(wv_sb[dc][:], wv[128 * dc:128 * dc + 128, :])
        # wo padded to K=128 with zero rows 64-127: fp32r matmuls with K=64
        # stream at ~2 cycles/row (HW-measured), K=128 at 1 -- zero-padding
        # the contraction nearly halves scores/Wo PE time. DMA'd after the
        # critical-path inputs (only needed in the Wo phase).
        wo_sb = const.tile([128, HG, D], MMDT, tag="wo")
        nc.sync.dma_start(wo_sb[0:DK, :, :], wo)
        for a in range(2):
            nc.vector.tensor_scalar_mul(
                wo_sb[DK:128, 2 * a:2 * a + 2, :],
                sin_sb[DK:128, :].rearrange("p (a b) -> p a b", a=2), 0.0)
        if with_bias:
            bq_sb = const.tile([1, NCOLS], MMDT, tag="bq")
            bk_sb = const.tile([1, NCOLS], MMDT, tag="bk")
            bv_sb = const.tile([1, NCOLS], MMDT, tag="bv")
            nc.sync.dma_start(bq_sb[:], bq)
            nc.sync.dma_start(bk_sb[:], bk)
            nc.sync.dma_start(bv_sb[:], bv)
        ones_row = const.tile([1, SB], MMDT, tag="ones_row")
        nc.sync.dma_start(ones_row[:], ones_row_d)
        onesc_sb = const.tile([128, DK], MMDT, tag="onesc")
        nc.sync.dma_start(onesc_sb[:], onesc_d)

        # Q^T / K^T per (chunk, sq-block): chunk c holds heads {2c, 2c+1}
        qt = [[sbig.tile([128, SB], MMDT, tag=f"qt{c}_{sb}", name=f"qt{c}_{sb}")
               for sb in range(NSB)] for c in range(2)]
        # per-head K^T, zero-padded to 128 partitions (head data on its chunk
        # rows, the complementary 64 rows zeroed)
        kth = [[sbig.tile([128, SB], MMDT, tag=f"kh{h}_{sb}", name=f"kh{h}_{sb}")
                for sb in range(NSB)] for h in range(HG)]
        for h in range(HG):
            zrows = slice(DK, 128) if h % 2 == 0 else slice(0, DK)
            for sb in range(NSB):
                nc.vector.tensor_scalar_mul(kth[h][sb][zrows, :],
                                            cos_sb[zrows, 0:SB], 0.0)
        # V augmented with a ones column per head, per key tile. Head stride
        # padded 65 -> 68 columns so each head's lhsT starts 16B-aligned.
        AUGW = DK + 4
        vaug = [sbig.tile([128, HG * AUGW], MMDT, tag=f"va{st}", name=f"va{st}")
                for st in range(NST)]
        # unnormalized O^T per (head, sq-block), zero-padded to 128 rows
        ot = [[sbig.tile([128, SB], MMDT, tag=f"ot{h}_{j}", name=f"ot{h}_{j}")
               for j in range(NSB)] for h in range(HG)]
        for h in range(HG):
            for j in range(NSB):
                nc.vector.tensor_scalar_mul(ot[h][j][DK:128, :],
                                             cos_sb[DK:128, 0:SB], 0.0)

        # ------------------------------------------------------- projections
        with tc.tile_pool(name="pj_ps", bufs=4, space="PSUM") as pj_ps, \
             tc.tile_pool(name="pv_ps", bufs=4, space="PSUM") as pvp_ps:
            for sb in range(NSB):
                ss = slice(SB * sb, SB * sb + SB)
                xt_t = []
                for dc in range(NDC):
                    t = xts.tile([128, SB], MMDT, tag="xt")
                    nc.sync.dma_start(t[:], xT[128 * dc:128 * dc + 128, ss])
                    xt_t.append(t)
                for c in range(2):
                    ncol = slice(128 * c, 128 * c + 128)
                    for (w_sb, bname) in ((wq_sb, "bq"), (wk_sb, "bk")):
                        ps = pj_ps.tile([128, SB], F32, tag="qk")
                        for dc in range(NDC):
                            nc.tensor.matmul(ps[:], w_sb[dc][:, ncol], xt_t[dc][:],
                                             start=(dc == 0),
                                             stop=(dc == NDC - 1 and not with_bias))
                        if with_bias:
                            b_sb = bq_sb if bname == "bq" else bk_sb
                            nc.tensor.matmul(ps[:], b_sb[0:1, ncol], ones_row[0:1, :],
                                             start=False, stop=True)
                        # rope: dst = ps*cos + shuffle(ps)*sin
                        t_cos = rtmp.tile([128, SB], F32, tag="rc")
                        nc.vector.tensor_mul(t_cos[:], ps[:], cos_sb[:, ss])
                        t_shuf = rtmp.tile([128, SB], F32, tag="rs")
                        nc.vector.stream_shuffle(t_shuf[:], ps[:], SHUF_MASK)
                        t_sin = rtmp.tile([128, SB], F32, tag="rm")
                        nc.gpsimd.tensor_mul(t_sin[:], t_shuf[:], sin_sb[:, ss])
                        if bname == "bq":
                            nc.vector.tensor_add(qt[c][sb][:], t_cos[:], t_sin[:])
                        else:
                            nc.vector.tensor_add(kth[2 * c][sb][0:DK, :],
                                                 t_cos[0:DK, :], t_sin[0:DK, :])
                            nc.vector.tensor_add(kth[2 * c + 1][sb][DK:128, :],
                                                 t_cos[DK:128, :], t_sin[DK:128, :])
                for st4 in range(SB // 128):
                    st = (SB // 128) * sb + st4
                    ps = pvp_ps.tile([128, NCOLS], F32, tag="v")
                    for dc in range(NDC):
                        nc.tensor.matmul(ps[:], xt_t[dc][:, 128 * st4:128 * st4 + 128],
                                         wv_sb[dc][:],
                                         start=(dc == 0),
                                         stop=(dc == NDC - 1 and not with_bias))
                    if with_bias:
                        nc.tensor.matmul(ps[:], ones_row[0:1, 0:128], bv_sb[0:1, :],
                                         start=False, stop=True)
                    # scatter heads into the augmented layout; even heads get
                    # [V | ones], odd heads [ones | V] (so PV psum offset 63
                    # puts their output on partitions 64-127)
                    va = vaug[st][:].rearrange("p (h e) -> p h e", h=HG)
                    nc.vector.tensor_copy(va[:, :, 0:DK],
                                          ps[:].rearrange("p (h k) -> p h k", h=HG))
                    nc.vector.tensor_copy(va[:, :, DK], onesc_sb[:, 0:HG])

        # -------------------------------------------------------- attention
        # S^T layout: psum group = GW key tiles x one sq block; exp on ACT;
        # PV accumulates (V | ones) so row 64 is the softmax denominator.
        with tc.tile_pool(name="sc_ps", bufs=2, space="PSUM") as sc_ps, \
             tc.tile_pool(name="o_ps", bufs=2, space="PSUM") as o_ps, \
             tc.tile_pool(name="bc_ps", bufs=2, space="PSUM") as bc_ps:
            for j in range(NSB):
                sq = slice(SB * j, SB * j + SB)
                for h in range(HG):
                    c, half = divmod(h, 2)
                    rows = slice(DK * half, DK * half + DK)
                    pv = o_ps.tile([128, SB], F32, tag="pv")
                    ngrp = (4 * j + 4) // GW
                    for g in range(ngrp):
                        sc = sc_ps.tile([128, GW * SB], F32, tag="sc")
                        for t in range(GW):
                            i = GW * g + t
                            nc.tensor.matmul(
                                sc[:, SB * t:SB * t + SB],
                                kth[h][i // 4][:, 128 * (i % 4):128 * (i % 4) + 128],
                                qt[c][j][:],
                                start=True, stop=True)
                        e = epool.tile([128, GW * SB], MMDT, tag="e")
                        nc.scalar.activation(e[:], sc[:],
                                             mybir.ActivationFunctionType.Exp,
                                             scale=SCALE)
                        d0 = GW * g - 4 * j
                        if d0 + GW > 0:  # group touches the causal diagonal
                            ev = e[:].rearrange("p (t f) -> p t f", t=GW)
                            nc.gpsimd.affine_select(
                                out=ev, in_=ev,
                                compare_op=mybir.AluOpType.is_ge,
                                fill=0.0, base=-128 * d0,
                                pattern=[[-128, GW], [1, SB]],
                                channel_multiplier=-1)
                        for t in range(GW):
                            i = GW * g + t
                            lhs = vaug[i][:].rearrange("p (h e) -> p h e", h=HG)[:, h, 0:DK + 1]
                            nc.tensor.matmul(
                                pv[0:DK + 1, :], lhs, e[:, SB * t:SB * t + SB],
                                start=(g == 0 and t == 0),
                                stop=(g == ngrp - 1 and t == GW - 1))
                    # normalize: ot = pv[0:64] * broadcast(1/pv[64])
                    rec = npool.tile([128, SB], MMDT, tag="rec")
                    with nc.allow_low_precision(reason="denominator recip in tf32"):
                        nc.vector.reciprocal(rec[DK:DK + 1, :], pv[DK:DK + 1, :])
                    bcp = bc_ps.tile([DK, SB], F32, tag="bc")
                    nc.tensor.matmul(bcp[:], onesc_sb[DK:DK + 1, :],
                                     rec[DK:DK + 1, :], start=True, stop=True)
                    bc = npool.tile([DK, SB], F32, tag="bcs")
                    nc.vector.tensor_copy(bc[:], bcp[:])
                    nc.vector.tensor_mul(ot[h][j][0:DK, :], pv[0:DK, :], bc[:])

        # ------------------------------------------------- output projection
        with tc.tile_pool(name="wo_ps", bufs=4, space="PSUM") as wo_ps:
            for st in range(NST):
                rq = slice(128 * (st % 4), 128 * (st % 4) + 128)
                jb = st // 4
                for dc in range(2):
                    cols = slice(SB * dc, SB * dc + SB)
                    ps = wo_ps.tile([128, SB], F32, tag="wo")
                    for h in range(HG):
                        nc.tensor.matmul(ps[:], ot[h][jb][:, rq], wo_sb[:, h, cols],
                                         start=(h == 0), stop=(h == HG - 1))
                    o_sb = opool.tile([128, SB], F32, tag="osb")
                    if (st + dc) % 2 == 0:
                        nc.vector.tensor_copy(o_sb[:], ps[:])
                    else:
                        nc.scalar.copy(o_sb[:], ps[:])
                    nc.sync.dma_start(out[128 * st:128 * st + 128, cols], o_sb[:])

    nc.compile()
    return nc


_CACHED_NC = {}


def _get_program(with_bias=False):
    if with_bias not in _CACHED_NC:
        _CACHED_NC[with_bias] = build_program(with_bias=with_bias)
    return _CACHED_NC[with_bias]


# ---------------------------------------------------------------------------
# entry point
# ---------------------------------------------------------------------------

def kernel(x, token_position, Wq, bq, Wk, bk, Wv, bv, Wo, bo, _results=None):
    from concourse.bass_utils import run_bass_kernel_spmd

    in_maps = make_core_inputs(x, token_position, Wq, bq, Wk, bk, Wv, bv, Wo, bo)
    if _results is None:
        with_bias = any(float(np.abs(np.asarray(v)).max()) != 0.0
                        for v in (bq, bk, bv))
        nc = _get_program(with_bias=with_bias)
        res = run_bass_kernel_spmd(nc, in_maps, list(range(N_CORES)))
        _results = [res.results[i]["out"] for i in range(N_CORES)]
    bo = np.asarray(bo, dtype=np.float32)
    out = np.empty((B, S, D), dtype=np.float32)
    for b in range(B):
        acc = _results[HG * b].astype(np.float32)
        for hg in range(1, HG):
            acc = acc + _results[HG * b + hg]
        out[b] = acc + bo[None, :]
    return out



# revision 4
# speedup vs baseline: 1.0333x; 1.0333x over previous
"""Trainium2 Bass kernel: causal multi-head attention with interleaved RoPE.

Problem shapes (hardcoded): x [2, 2048, 1024], 16 heads of dk=64.
Sharding: 8 cores = 2 batches x 4 head-groups (4 heads each). Each core
computes its head-slice Q/K/V projections, RoPE, causal attention, and a
partial output through its Wo row-slice; the host sums the 4 partials per
batch and adds bo.

RoPE trick: attention scores are invariant to any permutation of the dk
axis applied to both Q and K, so the Wq/Wk columns are permuted on the host
into a "quadrant half-split" layout where each rotation pair partner sits
exactly 16 partitions away inside the same 32-partition quadrant. The DVE
stream_shuffle (a per-quadrant 32-way permute) then produces the swapped
operand, and RoPE becomes: rot = q * cosT + shuffle(q) * sinT with
host-precomputed tables (sinT carries the sign).
"""

import os
from contextlib import ExitStack

import numpy as np

import concourse.bass as bass
import concourse.mybir as mybir
import concourse.tile as tile

B, S, D, H = 2, 2048, 1024, 16
DK = D // H  # 64
HG = 4  # heads per core
NCOLS = HG * DK  # 256 columns of the projection per core
THETA = 10000.0
SCALE = 1.0 / float(np.sqrt(DK))
N_CORES = 8

F32 = mybir.dt.float32
F32R = mybir.dt.float32r
BF16 = mybir.dt.bfloat16

# matmul operand dtype: bf16 streams 1 col/cycle at the full 2.4GHz PE clock
# (fp32r is SBUF-bandwidth limited to ~1.3GHz effective) and gets DVE 2x
# modes. Numerics validated in test.py (rel err ~4e-3, budget 2e-2).
MMDT = BF16


def round_tf32(a):
    """Round fp32 array to the matmul operand dtype (bf16)."""
    import ml_dtypes
    return np.ascontiguousarray(np.asarray(a, dtype=np.float32).astype(ml_dtypes.bfloat16))


# ---------------------------------------------------------------------------
# host-side prep
# ---------------------------------------------------------------------------

def _rope_perm():
    """Within-head column permutation pi: new row r -> original dk index."""
    perm = np.empty(DK, dtype=np.int64)
    for r in range(DK):
        q, m = divmod(r, 32)
        if m < 16:
            perm[r] = 2 * (16 * q + m)
        else:
            perm[r] = 2 * (16 * q + m - 16) + 1
    return perm


_PERM = _rope_perm()
SHUF_MASK = list(range(16, 32)) + list(range(16))  # swap 16-halves per quadrant


def _rope_tables(pos):
    """cosT/sinT [128, S] fp32 for the permuted layout. pos: [S] int."""
    inv_freq = (np.float32(THETA) ** (-(np.arange(0, DK, 2, dtype=np.float32) / np.float32(DK))))  # [32]
    ang = pos.astype(np.float32)[:, None] * inv_freq[None, :]  # [S, 32]
    cos = np.cos(ang)  # [S, 32]
    sin = np.sin(ang)
    cosT = np.empty((128, S), dtype=np.float32)
    sinT = np.empty((128, S), dtype=np.float32)
    for p in range(128):
        r = p % DK
        q, m = divmod(r, 32)
        if m < 16:
            i = 16 * q + m
            sgn = -1.0
        else:
            i = 16 * q + m - 16
            sgn = 1.0
        cosT[p] = cos[:, i]
        sinT[p] = np.float32(sgn) * sin[:, i]
    return cosT, sinT


def make_core_inputs(x, token_position, Wq, bq, Wk, bk, Wv, bv, Wo, bo):
    """Build the 8 per-core input maps."""
    x = np.asarray(x, dtype=np.float32)
    token_position = np.asarray(token_position)
    Wq, Wk, Wv, Wo = (np.asarray(w, dtype=np.float32) for w in (Wq, Wk, Wv, Wo))
    bq, bk, bv = (np.asarray(b_, dtype=np.float32) for b_ in (bq, bk, bv))

    in_maps = []
    tables = {}
    for c in range(N_CORES):
        b, hg = divmod(c, HG)
        heads = range(HG * hg, HG * hg + HG)
        # permuted q/k column indices for this core's heads
        cols_qk = np.concatenate([DK * h + _PERM for h in heads])
        cols_v = np.arange(NCOLS * hg, NCOLS * hg + NCOLS)
        if b not in tables:
            tables[b] = _rope_tables(np.asarray(token_position[b]))
        cosT, sinT = tables[b]
        wo_rows = Wo[cols_v, :]  # [256, 1024]
        in_maps.append({
            "xT": round_tf32(x[b].T),                               # [1024, 2048]
            "wq": round_tf32(Wq[:, cols_qk]),                       # [1024, 256]
            "wk": round_tf32(Wk[:, cols_qk]),
            "wv": round_tf32(Wv[:, cols_v]),
            "wo": round_tf32(wo_rows.reshape(HG, DK, D).transpose(1, 0, 2)),  # [64, 4, 1024]
            "bq": round_tf32(bq[cols_qk][None, :]),                 # [1, 256]
            "bk": round_tf32(bk[cols_qk][None, :]),
            "bv": round_tf32(bv[cols_v][None, :]),
            "ones_row": round_tf32(np.ones((1, 512), np.float32)),
            "onesc": round_tf32(np.ones((128, 64), np.float32)),
            "cosT": cosT,
            "sinT": sinT,
        })
    return in_maps


# ---------------------------------------------------------------------------
# device program
# ---------------------------------------------------------------------------

def build_program(with_bias=False):
    from concourse import bacc, library_config
    nc = bacc.Bacc("TRN2", debug=False)

    xT = nc.declare_dram_parameter("xT", [D, S], MMDT, isOutput=False).ap()
    wq = nc.declare_dram_parameter("wq", [D, NCOLS], MMDT, isOutput=False).ap()
    wk = nc.declare_dram_parameter("wk", [D, NCOLS], MMDT, isOutput=False).ap()
    wv = nc.declare_dram_parameter("wv", [D, NCOLS], MMDT, isOutput=False).ap()
    wo = nc.declare_dram_parameter("wo", [DK, HG, D], MMDT, isOutput=False).ap()
    bq = nc.declare_dram_parameter("bq", [1, NCOLS], MMDT, isOutput=False).ap()
    bk = nc.declare_dram_parameter("bk", [1, NCOLS], MMDT, isOutput=False).ap()
    bv = nc.declare_dram_parameter("bv", [1, NCOLS], MMDT, isOutput=False).ap()
    ones_row_d = nc.declare_dram_parameter("ones_row", [1, 512], MMDT, isOutput=False).ap()
    onesc_d = nc.declare_dram_parameter("onesc", [128, DK], MMDT, isOutput=False).ap()
    cosT = nc.declare_dram_parameter("cosT", [128, S], F32, isOutput=False).ap()
    sinT = nc.declare_dram_parameter("sinT", [128, S], F32, isOutput=False).ap()
    out = nc.declare_dram_parameter("out", [S, D], F32, isOutput=True).ap()

    SB = 512            # sq block width
    NSB = S // SB       # 4
    NST = S // 128      # 16 key tiles / V tiles
    NDC = D // 128      # 8 contraction chunks
    GW = 2              # key tiles per score-psum group

    with tile.TileContext(nc) as tc, ExitStack() as ctx:
        nc.gpsimd.load_library(library_config.proxy)
        const = ctx.enter_context(tc.tile_pool(name="const", bufs=1))
        sbig = ctx.enter_context(tc.tile_pool(name="sbig", bufs=1))
        xts = ctx.enter_context(tc.tile_pool(name="xts", bufs=4))
        rtmp = ctx.enter_context(tc.tile_pool(name="rtmp", bufs=2))
        epool = ctx.enter_context(tc.tile_pool(name="epool", bufs=3))
        npool = ctx.enter_context(tc.tile_pool(name="npool", bufs=3))
        opool = ctx.enter_context(tc.tile_pool(name="opool", bufs=2))

        # --- constants / weights resident in SBUF (per-dc tiles: finer deps,
        # so the first projection matmuls start after ~128KB of DMA)
        wq_sb = [const.tile([128, NCOLS], MMDT, tag=f"wq{dc}", name=f"wq{dc}")
                 for dc in range(NDC)]
        wk_sb = [const.tile([128, NCOLS], MMDT, tag=f"wk{dc}", name=f"wk{dc}")
                 for dc in range(NDC)]
        wv_sb = [const.tile([128, NCOLS], MMDT, tag=f"wv{dc}", name=f"wv{dc}")
                 for dc in range(NDC)]
        for dc in range(NDC):
            nc.sync.dma_start(wq_sb[dc][:], wq[128 * dc:128 * dc + 128, :])
            nc.sync.dma_start(wk_sb[dc][:], wk[128 * dc:128 * dc + 128, :])
        cos_sb = const.tile([128, S], F32, tag="cos")
        sin_sb = const.tile([128, S], F32, tag="sin")
        nc.sync.dma_start(cos_sb[:], cosT)
        nc.sync.dma_start(sin_sb[:], sinT)
        for dc in range(NDC):
            nc.sync.dma_start(wv_sb[dc][:], wv[128 * dc:128 * dc + 128, :])
        # wo padded to K=128 with zero rows 64-127: fp32r matmuls with K=64
        # stream at ~2 cycles/row (HW-measured), K=128 at 1 -- zero-padding
        # the contraction nearly halves scores/Wo PE time. DMA'd after the
        # critical-path inputs (only needed in the Wo phase).
        wo_sb = const.tile([128, HG, D], MMDT, tag="wo")
        nc.sync.dma_start(wo_sb[0:DK, :, :], wo)
        for a in range(2):
            nc.vector.tensor_scalar_mul(
                wo_sb[DK:128, 2 * a:2 * a + 2, :],
                sin_sb[DK:128, :].rearrange("p (a b) -> p a b", a=2), 0.0)
        if with_bias:
            bq_sb = const.tile([1, NCOLS], MMDT, tag="bq")
            bk_sb = const.tile([1, NCOLS], MMDT, tag="bk")
            bv_sb = const.tile([1, NCOLS], MMDT, tag="bv")
            nc.sync.dma_start(bq_sb[:], bq)
            nc.sync.dma_start(bk_sb[:], bk)
            nc.sync.dma_start(bv_sb[:], bv)
        ones_row = const.tile([1, SB], MMDT, tag="ones_row")
        nc.sync.dma_start(ones_row[:], ones_row_d)
        onesc_sb = const.tile([128, DK], MMDT, tag="onesc")
        nc.sync.dma_start(onesc_sb[:], onesc_d)

        # Q^T / K^T per (chunk, sq-block): chunk c holds heads {2c, 2c+1}
        qt = [[sbig.tile([128, SB], MMDT, tag=f"qt{c}_{sb}", name=f"qt{c}_{sb}")
               for sb in range(NSB)] for c in range(2)]
        # per-head K^T, zero-padded to 128 partitions (head data on its chunk
        # rows, the complementary 64 rows zeroed)
        kth = [[sbig.tile([128, SB], MMDT, tag=f"kh{h}_{sb}", name=f"kh{h}_{sb}")
                for sb in range(NSB)] for h in range(HG)]
        for h in range(HG):
            zrows = slice(DK, 128) if h % 2 == 0 else slice(0, DK)
            for sb in range(NSB):
                nc.vector.tensor_scalar_mul(kth[h][sb][zrows, :],
                                            cos_sb[zrows, 0:SB], 0.0)
        # V augmented with a ones column per head, per key tile. Head stride
        # padded 65 -> 68 columns so each head's lhsT starts 16B-aligned.
        AUGW = DK + 4
        vaug = [sbig.tile([128, HG * AUGW], MMDT, tag=f"va{st}", name=f"va{st}")
                for st in range(NST)]
        # unnormalized O^T per (head, sq-block), zero-padded to 128 rows
        ot = [[sbig.tile([128, SB], MMDT, tag=f"ot{h}_{j}", name=f"ot{h}_{j}")
               for j in range(NSB)] for h in range(HG)]
        for h in range(HG):
            for j in range(NSB):
                nc.vector.tensor_scalar_mul(ot[h][j][DK:128, :],
                                             cos_sb[DK:128, 0:SB], 0.0)

        # ------------------------------------------------------- projections
        with tc.tile_pool(name="pj_ps", bufs=4, space="PSUM") as pj_ps, \
             tc.tile_pool(name="pv_ps", bufs=4, space="PSUM") as pvp_ps:
            for sb in range(NSB):
                ss = slice(SB * sb, SB * sb + SB)
                xt_t = []
                for dc in range(NDC):
                    t = xts.tile([128, SB], MMDT, tag="xt")
                    nc.sync.dma_start(t[:], xT[128 * dc:128 * dc + 128, ss])
                    xt_t.append(t)
                for c in range(2):
                    ncol = slice(128 * c, 128 * c + 128)
                    for (w_sb, bname) in ((wq_sb, "bq"), (wk_sb, "bk")):
                        ps = pj_ps.tile([128, SB], F32, tag="qk")
                        for dc in range(NDC):
                            nc.tensor.matmul(ps[:], w_sb[dc][:, ncol], xt_t[dc][:],
                                             start=(dc == 0),
                                             stop=(dc == NDC - 1 and not with_bias))
                        if with_bias:
                            b_sb = bq_sb if bname == "bq" else bk_sb
                            nc.tensor.matmul(ps[:], b_sb[0:1, ncol], ones_row[0:1, :],
                                             start=False, stop=True)
                        # rope: dst = ps*cos + shuffle(ps)*sin
                        t_cos = rtmp.tile([128, SB], F32, tag="rc")
                        nc.vector.tensor_mul(t_cos[:], ps[:], cos_sb[:, ss])
                        t_shuf = rtmp.tile([128, SB], F32, tag="rs")
                        nc.vector.stream_shuffle(t_shuf[:], ps[:], SHUF_MASK)
                        t_sin = rtmp.tile([128, SB], F32, tag="rm")
                        nc.gpsimd.tensor_mul(t_sin[:], t_shuf[:], sin_sb[:, ss])
                        if bname == "bq":
                            nc.vector.tensor_add(qt[c][sb][:], t_cos[:], t_sin[:])
                        else:
                            nc.vector.tensor_add(kth[2 * c][sb][0:DK, :],
                                                 t_cos[0:DK, :], t_sin[0:DK, :])
                            nc.vector.tensor_add(kth[2 * c + 1][sb][DK:128, :],
                                                 t_cos[DK:128, :], t_sin[DK:128, :])
                for st4 in range(SB // 128):
                    st = (SB // 128) * sb + st4
                    ps = pvp_ps.tile([128, NCOLS], F32, tag="v")
                    for dc in range(NDC):
                        nc.tensor.matmul(ps[:], xt_t[dc][:, 128 * st4:128 * st4 + 128],
                                         wv_sb[dc][:],
                                         start=(dc == 0),
                                         stop=(dc == NDC - 1 and not with_bias))
                    if with_bias:
                        nc.tensor.matmul(ps[:], ones_row[0:1, 0:128], bv_sb[0:1, :],
                                         start=False, stop=True)
                    # scatter heads into the augmented layout; even heads get
                    # [V | ones], odd heads [ones | V] (so PV psum offset 63
                    # puts their output on partitions 64-127)
                    va = vaug[st][:].rearrange("p (h e) -> p h e", h=HG)
                    nc.vector.tensor_copy(va[:, :, 0:DK],
                                          ps[:].rearrange("p (h k) -> p h k", h=HG))
                    nc.vector.tensor_copy(va[:, :, DK], onesc_sb[:, 0:HG])

        # -------------------------------------------------------- attention
        # S^T layout: psum group = GW key tiles x one sq block; exp on ACT;
        # PV accumulates (V | ones) so row 64 is the softmax denominator.
        with tc.tile_pool(name="sc_ps", bufs=2, space="PSUM") as sc_ps, \
             tc.tile_pool(name="o_ps", bufs=2, space="PSUM") as o_ps, \
             tc.tile_pool(name="bc_ps", bufs=2, space="PSUM") as bc_ps:
            for j in range(NSB):
                sq = slice(SB * j, SB * j + SB)
                for h in range(HG):
                    c, half = divmod(h, 2)
                    rows = slice(DK * half, DK * half + DK)
                    pv = o_ps.tile([128, SB], F32, tag="pv")
                    ngrp = (4 * j + 4) // GW
                    for g in range(ngrp):
                        sc = sc_ps.tile([128, GW * SB], F32, tag="sc")
                        for t in range(GW):
                            i = GW * g + t
                            nc.tensor.matmul(
                                sc[:, SB * t:SB * t + SB],
                                kth[h][i // 4][:, 128 * (i % 4):128 * (i % 4) + 128],
                                qt[c][j][:],
                                start=True, stop=True)
                        e = epool.tile([128, GW * SB], MMDT, tag="e")
                        nc.scalar.activation(e[:], sc[:],
                                             mybir.ActivationFunctionType.Exp,
                                             scale=SCALE)
                        d0 = GW * g - 4 * j
                        if d0 + GW > 0:  # group touches the causal diagonal
                            ev = e[:].rearrange("p (t f) -> p t f", t=GW)
                            nc.gpsimd.affine_select(
                                out=ev, in_=ev,
                                compare_op=mybir.AluOpType.is_ge,
                                fill=0.0, base=-128 * d0,
                                pattern=[[-128, GW], [1, SB]],
                                channel_multiplier=-1)
                        for t in range(GW):
                            i = GW * g + t
                            lhs = vaug[i][:].rearrange("p (h e) -> p h e", h=HG)[:, h, 0:DK + 1]
                            nc.tensor.matmul(
                                pv[0:DK + 1, :], lhs, e[:, SB * t:SB * t + SB],
                                start=(g == 0 and t == 0),
                                stop=(g == ngrp - 1 and t == GW - 1))
                    # normalize: ot = pv[0:64] * broadcast(1/pv[64])
                    rec = npool.tile([128, SB], MMDT, tag="rec")
                    with nc.allow_low_precision(reason="denominator recip in tf32"):
                        nc.vector.reciprocal(rec[DK:DK + 1, :], pv[DK:DK + 1, :])
                    bcp = bc_ps.tile([DK, SB], F32, tag="bc")
                    nc.tensor.matmul(bcp[:], onesc_sb[DK:DK + 1, :],
                                     rec[DK:DK + 1, :], start=True, stop=True)
                    bc = npool.tile([DK, SB], F32, tag="bcs")
                    nc.vector.tensor_copy(bc[:], bcp[:])
                    nc.vector.tensor_mul(ot[h][j][0:DK, :], pv[0:DK, :], bc[:])

        # ------------------------------------------------- output projection
        with tc.tile_pool(name="wo_ps", bufs=4, space="PSUM") as wo_ps:
            for st in range(NST):
                rq = slice(128 * (st % 4), 128 * (st % 4) + 128)
                jb = st // 4
                for dc in range(2):
                    cols = slice(SB * dc, SB * dc + SB)
                    ps = wo_ps.tile([128, SB], F32, tag="wo")
                    for h in range(HG):
                        nc.tensor.matmul(ps[:], ot[h][jb][:, rq], wo_sb[:, h, cols],
                                         start=(h == 0), stop=(h == HG - 1))
                    o_sb = opool.tile([128, SB], F32, tag="osb")
                    if (st + dc) % 2 == 0:
                        nc.vector.tensor_copy(o_sb[:], ps[:])
                    else:
                        nc.scalar.copy(o_sb[:], ps[:])
                    nc.sync.dma_start(out[128 * st:128 * st + 128, cols], o_sb[:])

    nc.compile()
    return nc


_CACHED_NC = {}


def _get_program(with_bias=False):
    if with_bias not in _CACHED_NC:
        _CACHED_NC[with_bias] = build_program(with_bias=with_bias)
    return _CACHED_NC[with_bias]


# ---------------------------------------------------------------------------
# entry point
# ---------------------------------------------------------------------------

def kernel(x, token_position, Wq, bq, Wk, bk, Wv, bv, Wo, bo, _results=None):
    from concourse.bass_utils import run_bass_kernel_spmd

    in_maps = make_core_inputs(x, token_position, Wq, bq, Wk, bk, Wv, bv, Wo, bo)
    if _results is None:
        with_bias = any(float(np.abs(np.asarray(v)).max()) != 0.0
                        for v in (bq, bk, bv))
        nc = _get_program(with_bias=with_bias)
        res = run_bass_kernel_spmd(nc, in_maps, list(range(N_CORES)))
        _results = [res.results[i]["out"] for i in range(N_CORES)]
    bo = np.asarray(bo, dtype=np.float32)
    out = np.empty((B, S, D), dtype=np.float32)
    for b in range(B):
        acc = _results[HG * b].astype(np.float32)
        for hg in range(1, HG):
            acc = acc + _results[HG * b + hg]
        out[b] = acc + bo[None, :]
    return out



# revision 25
# speedup vs baseline: 1.1595x; 1.1221x over previous
"""Trainium2 Bass kernel: causal multi-head attention with interleaved RoPE.

Problem shapes (hardcoded): x [2, 2048, 1024], 16 heads of dk=64.
Sharding: 8 cores = 2 batches x 4 head-groups (4 heads each). Each core
computes its head-slice Q/K/V projections, RoPE, causal attention, and a
partial output through its Wo row-slice; the host sums the 4 partials per
batch and adds bo.

All matmul operands are fp16 (1 col/cycle at the full 2.4GHz PE clock;
fp32r is SBUF-bandwidth limited to ~1.3GHz effective). PSUM accumulation
stays fp32. End-to-end rel err ~6e-4 (budget 2e-2).

RoPE trick: attention scores are invariant to any permutation of the dk
axis applied to both Q and K, so the Wq/Wk columns are permuted on the host
into a "quadrant half-split" layout where each rotation pair partner sits
exactly 16 partitions away inside the same 32-partition quadrant. The DVE
stream_shuffle (a per-quadrant 32-way permute) then produces the swapped
operand, and RoPE becomes: rot = q * cosT + shuffle(q) * sinT with
host-precomputed tables (sinT carries the sign).

Phase order per PE queue: proj0, proj1, attn0, proj2, wo0, attn1, proj3,
wo1, attn2, wo2, attn3, wo3 — keeps the PE stream dependency-slack ahead
of the DVE/ACT producers (rope, exp, normalize) feeding it.
"""

import os
from contextlib import ExitStack

import numpy as np

import concourse.bass as bass
import concourse.mybir as mybir
import concourse.tile as tile

B, S, D, H = 2, 2048, 1024, 16
DK = D // H  # 64
HG = 4  # heads per core
NCOLS = HG * DK  # 256 columns of the projection per core
THETA = 10000.0
SCALE = 1.0 / float(np.sqrt(DK))
N_CORES = 8

F32 = mybir.dt.float32
F32R = mybir.dt.float32r
F16 = mybir.dt.float16
MMDT = F16


def to_f16(a):
    return np.ascontiguousarray(np.asarray(a, dtype=np.float32).astype(np.float16))


# ---------------------------------------------------------------------------
# host-side prep
# ---------------------------------------------------------------------------

def _rope_perm():
    """Within-head column permutation pi: new row r -> original dk index."""
    perm = np.empty(DK, dtype=np.int64)
    for r in range(DK):
        q, m = divmod(r, 32)
        if m < 16:
            perm[r] = 2 * (16 * q + m)
        else:
            perm[r] = 2 * (16 * q + m - 16) + 1
    return perm


_PERM = _rope_perm()
SHUF_MASK = list(range(16, 32)) + list(range(16))  # swap 16-halves per quadrant


def _rope_tables(pos):
    """cosT/sinT [128, S] fp32 for the permuted layout. pos: [S] int."""
    inv_freq = (np.float32(THETA) ** (-(np.arange(0, DK, 2, dtype=np.float32) / np.float32(DK))))  # [32]
    ang = pos.astype(np.float32)[:, None] * inv_freq[None, :]  # [S, 32]
    cos = np.cos(ang)  # [S, 32]
    sin = np.sin(ang)
    cosT = np.empty((128, S), dtype=np.float32)
    sinT = np.empty((128, S), dtype=np.float32)
    for p in range(128):
        r = p % DK
        q, m = divmod(r, 32)
        if m < 16:
            i = 16 * q + m
            sgn = -1.0
        else:
            i = 16 * q + m - 16
            sgn = 1.0
        cosT[p] = cos[:, i]
        sinT[p] = np.float32(sgn) * sin[:, i]
    return cosT, sinT


def make_core_inputs(x, token_position, Wq, bq, Wk, bk, Wv, bv, Wo, bo):
    """Build the 8 per-core input maps."""
    x = np.asarray(x, dtype=np.float32)
    token_position = np.asarray(token_position)
    Wq, Wk, Wv, Wo = (np.asarray(w, dtype=np.float32) for w in (Wq, Wk, Wv, Wo))
    bq, bk, bv = (np.asarray(b_, dtype=np.float32) for b_ in (bq, bk, bv))

    in_maps = []
    tables = {}
    for c in range(N_CORES):
        b, hg = divmod(c, HG)
        heads = range(HG * hg, HG * hg + HG)
        # permuted q/k column indices for this core's heads
        cols_qk = np.concatenate([DK * h + _PERM for h in heads])
        cols_v = np.arange(NCOLS * hg, NCOLS * hg + NCOLS)
        if b not in tables:
            tables[b] = _rope_tables(np.asarray(token_position[b]))
        cosT, sinT = tables[b]
        wo_rows = Wo[cols_v, :]  # [256, 1024] (head-major rows)
        # pair-packed Wo: lane l of pair p holds Wo row of head 2p + l//64,
        # dk l%64 -- so one K=128 matmul contracts a full head pair.
        wo_packed = np.empty((128, 2, D), dtype=np.float32)
        for l_ in range(128):
            for p in range(2):
                h = 2 * p + l_ // 64
                wo_packed[l_, p, :] = wo_rows[DK * h + (l_ % 64), :]
        in_maps.append({
            "xT": to_f16(x[b].T),                               # [1024, 2048]
            "wq": to_f16(Wq[:, cols_qk]),                       # [1024, 256]
            "wk": to_f16(Wk[:, cols_qk]),
            "wv": to_f16(Wv[:, cols_v]),
            "wo": to_f16(wo_packed),                            # [128, 2, 1024]
            "bq": to_f16(bq[cols_qk][None, :]),                 # [1, 256]
            "bk": to_f16(bk[cols_qk][None, :]),
            "bv": to_f16(bv[cols_v][None, :]),
            "ones_row": to_f16(np.ones((1, 512), np.float32)),
            "onesc": to_f16(np.ones((128, DK), np.float32)),
            "cosT": cosT,
            "sinT": sinT,
        })
    return in_maps


# ---------------------------------------------------------------------------
# device program
# ---------------------------------------------------------------------------

def build_program(with_bias=False):
    from concourse import bacc, library_config
    nc = bacc.Bacc("TRN2", debug=False)

    xT = nc.declare_dram_parameter("xT", [D, S], MMDT, isOutput=False).ap()
    wq = nc.declare_dram_parameter("wq", [D, NCOLS], MMDT, isOutput=False).ap()
    wk = nc.declare_dram_parameter("wk", [D, NCOLS], MMDT, isOutput=False).ap()
    wv = nc.declare_dram_parameter("wv", [D, NCOLS], MMDT, isOutput=False).ap()
    wo = nc.declare_dram_parameter("wo", [128, 2, D], MMDT, isOutput=False).ap()
    bq = nc.declare_dram_parameter("bq", [1, NCOLS], MMDT, isOutput=False).ap()
    bk = nc.declare_dram_parameter("bk", [1, NCOLS], MMDT, isOutput=False).ap()
    bv = nc.declare_dram_parameter("bv", [1, NCOLS], MMDT, isOutput=False).ap()
    ones_row_d = nc.declare_dram_parameter("ones_row", [1, 512], MMDT, isOutput=False).ap()
    onesc_d = nc.declare_dram_parameter("onesc", [128, DK], MMDT, isOutput=False).ap()
    cosT = nc.declare_dram_parameter("cosT", [128, S], F32, isOutput=False).ap()
    sinT = nc.declare_dram_parameter("sinT", [128, S], F32, isOutput=False).ap()
    out = nc.declare_dram_parameter("out", [S, D], F16, isOutput=True).ap()

    SB = 512            # sq block width
    NSB = S // SB       # 4
    NST = S // 128      # 16 key tiles / V tiles
    NDC = D // 128      # 8 contraction chunks
    GW = 2              # key tiles per score-psum group
    AUGW = DK + 8       # V head stride (72): 16B-aligned fp16 lhsT starts

    with tile.TileContext(nc) as tc, ExitStack() as ctx:
        nc.gpsimd.load_library(library_config.proxy)
        const = ctx.enter_context(tc.tile_pool(name="const", bufs=1))
        sbig = ctx.enter_context(tc.tile_pool(name="sbig", bufs=1))
        xts = ctx.enter_context(tc.tile_pool(name="xts", bufs=8))
        rtmp = ctx.enter_context(tc.tile_pool(name="rtmp", bufs=2))
        epool = ctx.enter_context(tc.tile_pool(name="epool", bufs=3))
        npool = ctx.enter_context(tc.tile_pool(name="npool", bufs=3))
        opool = ctx.enter_context(tc.tile_pool(name="opool", bufs=4))
        mm_ps = ctx.enter_context(tc.tile_pool(name="mm_ps", bufs=2, space="PSUM"))
        sc_ps = ctx.enter_context(tc.tile_pool(name="sc_ps", bufs=2, space="PSUM"))
        pv_ps = ctx.enter_context(tc.tile_pool(name="pv_ps", bufs=2, space="PSUM"))

        # --- constants / weights resident in SBUF (per-dc tiles: finer deps,
        # so the first projection matmuls start early)
        wq_sb = [const.tile([128, NCOLS], MMDT, tag=f"wq{dc}", name=f"wq{dc}")
                 for dc in range(NDC)]
        wk_sb = [const.tile([128, NCOLS], MMDT, tag=f"wk{dc}", name=f"wk{dc}")
                 for dc in range(NDC)]
        wv_sb = [const.tile([128, NCOLS], MMDT, tag=f"wv{dc}", name=f"wv{dc}")
                 for dc in range(NDC)]
        for dc in range(NDC):
            nc.sync.dma_start(wq_sb[dc][:], wq[128 * dc:128 * dc + 128, :])
            nc.sync.dma_start(wk_sb[dc][:], wk[128 * dc:128 * dc + 128, :])
        cos_sb = const.tile([128, S], F32, tag="cos")
        sin_sb = const.tile([128, S], F32, tag="sin")
        nc.sync.dma_start(cos_sb[:], cosT)
        nc.sync.dma_start(sin_sb[:], sinT)
        for dc in range(NDC):
            nc.sync.dma_start(wv_sb[dc][:], wv[128 * dc:128 * dc + 128, :])
        wo_sb = const.tile([128, 2, D], MMDT, tag="wo")
        nc.sync.dma_start(wo_sb[:], wo)
        if with_bias:
            bq_sb = const.tile([1, NCOLS], MMDT, tag="bq")
            bk_sb = const.tile([1, NCOLS], MMDT, tag="bk")
            bv_sb = const.tile([1, NCOLS], MMDT, tag="bv")
            nc.sync.dma_start(bq_sb[:], bq)
            nc.sync.dma_start(bk_sb[:], bk)
            nc.sync.dma_start(bv_sb[:], bv)
        ones_row = const.tile([1, SB], MMDT, tag="ones_row")
        nc.sync.dma_start(ones_row[:], ones_row_d)
        onesc_sb = const.tile([128, DK], MMDT, tag="onesc")
        nc.sync.dma_start(onesc_sb[:], onesc_d)

        # Q^T per (chunk, sq-block): chunk c holds heads {2c, 2c+1}
        qt = [[sbig.tile([128, SB], MMDT, tag=f"qt{c}_{sb}", name=f"qt{c}_{sb}")
               for sb in range(NSB)] for c in range(2)]
        # per-head K^T, zero-padded to 128 partitions (head data on its chunk
        # rows, the complementary 64 rows zeroed)
        kth = [[sbig.tile([128, SB], MMDT, tag=f"kh{h}_{sb}", name=f"kh{h}_{sb}")
                for sb in range(NSB)] for h in range(HG)]
        for h in range(HG):
            zrows = slice(DK, 128) if h % 2 == 0 else slice(0, DK)
            for sb in range(NSB):
                nc.vector.tensor_scalar_mul(kth[h][sb][zrows, :],
                                            cos_sb[zrows, 0:SB], 0.0)
        # V augmented with a ones column per head, per key tile.
        vaug = [sbig.tile([128, HG * AUGW], MMDT, tag=f"va{st}", name=f"va{st}")
                for st in range(NST)]
        # normalized O^T per (head-pair, sq-block): lanes 0:64 = even head's
        # dk, 64:128 = odd head's dk -- Wo contracts a dense K=128 per pair.
        ot = [[sbig.tile([128, SB], MMDT, tag=f"ot{p}_{j}", name=f"ot{p}_{j}")
               for j in range(NSB)] for p in range(2)]

        ncopy = [0]

        def out_copy(dst, src):
            k = ncopy[0] % 2
            ncopy[0] += 1
            if k == 0:
                nc.vector.tensor_copy(dst, src)
            else:
                nc.scalar.copy(dst, src)

        def proj(sb):
            ss = slice(SB * sb, SB * sb + SB)
            xt_t = []
            for dc in range(NDC):
                t = xts.tile([128, SB], MMDT, tag="xt")
                nc.sync.dma_start(t[:], xT[128 * dc:128 * dc + 128, ss])
                xt_t.append(t)
            for c in range(2):
                ncol = slice(128 * c, 128 * c + 128)
                for (w_sb, bname) in ((wq_sb, "bq"), (wk_sb, "bk")):
                    ps = mm_ps.tile([128, SB], F32, tag="mm")
                    for dc in range(NDC):
                        nc.tensor.matmul(ps[:], w_sb[dc][:, ncol], xt_t[dc][:],
                                         start=(dc == 0),
                                         stop=(dc == NDC - 1 and not with_bias))
                    if with_bias:
                        b_sb = bq_sb if bname == "bq" else bk_sb
                        nc.tensor.matmul(ps[:], b_sb[0:1, ncol], ones_row[0:1, :],
                                         start=False, stop=True)
                    # rope: dst = ps*cos + shuffle(ps)*sin
                    t_cos = rtmp.tile([128, SB], F32, tag="rc")
                    nc.vector.tensor_mul(t_cos[:], ps[:], cos_sb[:, ss])
                    t_shuf = rtmp.tile([128, SB], F32, tag="rs")
                    nc.vector.stream_shuffle(t_shuf[:], ps[:], SHUF_MASK)
                    t_sin = rtmp.tile([128, SB], F32, tag="rm")
                    nc.gpsimd.tensor_mul(t_sin[:], t_shuf[:], sin_sb[:, ss])
                    if bname == "bq":
                        nc.vector.tensor_add(qt[c][sb][:], t_cos[:], t_sin[:])
                    else:
                        nc.vector.tensor_add(kth[2 * c][sb][0:DK, :],
                                             t_cos[0:DK, :], t_sin[0:DK, :])
                        nc.vector.tensor_add(kth[2 * c + 1][sb][DK:128, :],
                                             t_cos[DK:128, :], t_sin[DK:128, :])
            for st4 in range(SB // 128):
                st = (SB // 128) * sb + st4
                ps = mm_ps.tile([128, SB], F32, tag="mm")
                for dc in range(NDC):
                    nc.tensor.matmul(ps[:, 0:NCOLS],
                                     xt_t[dc][:, 128 * st4:128 * st4 + 128],
                                     wv_sb[dc][:],
                                     start=(dc == 0),
                                     stop=(dc == NDC - 1 and not with_bias))
                if with_bias:
                    nc.tensor.matmul(ps[:, 0:NCOLS], ones_row[0:1, 0:128],
                                     bv_sb[0:1, :], start=False, stop=True)
                va = vaug[st][:].rearrange("p (h e) -> p h e", h=HG)
                nc.vector.tensor_copy(va[:, :, 0:DK],
                                      ps[:, 0:NCOLS].rearrange("p (h k) -> p h k", h=HG))
                nc.vector.tensor_copy(va[:, :, DK], onesc_sb[:, 0:HG])

        def attn(j):
            # S^T layout: psum group = GW key tiles x one sq block; exp on ACT
            # over the causally-valid column ranges only; PV accumulates
            # (V | ones) so row 64 is the softmax denominator.

            for h in range(HG):
                c, half = divmod(h, 2)
                pv = pv_ps.tile([128, SB], F32, tag="pv")
                ngrp = (4 * j + 4) // GW
                for g in range(ngrp):
                    sc = sc_ps.tile([128, GW * SB], F32, tag="sc")
                    # lo[t]: first causally-valid query column for key tile
                    # GW*g + t; scores/exp/PV all skip cols below it.
                    los = [min(max(128 * (GW * g + t - 4 * j), 0), SB)
                           for t in range(GW)]
                    for t in range(GW):
                        i = GW * g + t
                        lo = los[t]
                        if lo >= SB:
                            continue
                        nc.tensor.matmul(
                            sc[:, SB * t + lo:SB * t + SB],
                            kth[h][i // 4][:, 128 * (i % 4):128 * (i % 4) + 128],
                            qt[c][j][:, lo:SB],
                            start=True, stop=True)
                    e = epool.tile([128, GW * SB], MMDT, tag="e")
                    diag = GW * g + GW - 4 * j > 0
                    for t in range(GW):
                        lo = los[t]
                        if lo > 0:
                            # zero the causally-dead prefix: PV streams the
                            # full e width (uniform psum accumulation region)
                            nc.vector.memset(e[:, SB * t:SB * t + lo], 0.0)
                        nc.scalar.activation(
                            e[:, SB * t + lo:SB * t + SB],
                            sc[:, SB * t + lo:SB * t + SB],
                            mybir.ActivationFunctionType.Exp,
                            scale=SCALE)
                        if diag and 128 * (GW * g + t - 4 * j) >= 0:
                            # triangle band: zero e where query < key
                            band = e[:, SB * t + lo:SB * t + lo + 128]
                            nc.gpsimd.affine_select(
                                out=band.rearrange("p (o f) -> p o f", o=1),
                                in_=band.rearrange("p (o f) -> p o f", o=1),
                                compare_op=mybir.AluOpType.is_ge,
                                fill=0.0, base=0,
                                pattern=[[-128, 1], [1, 128]],
                                channel_multiplier=-1)
                    for t in range(GW):
                        i = GW * g + t
                        lhs = vaug[i][:].rearrange("p (h e) -> p h e", h=HG)[:, h, 0:DK + 1]
                        nc.tensor.matmul(
                            pv[0:DK + 1, :], lhs, e[:, SB * t:SB * t + SB],
                            start=(g == 0 and t == 0),
                            stop=(g == ngrp - 1 and t == GW - 1))
                # normalize: 1/den = exp(-ln(den)) on ACT (ln/exp/copy share
                # one activation table -> no table reloads; rel err ~1e-5),
                # PE broadcast, then scale the head's 64 output dims
                p, u = divmod(h, 2)
                lnden = npool.tile([128, SB], F32, tag="lnden")
                nc.scalar.activation(lnden[DK:DK + 1, :], pv[DK:DK + 1, :],
                                     mybir.ActivationFunctionType.Ln)
                rec16 = npool.tile([128, SB], MMDT, tag="rec16")
                nc.scalar.activation(rec16[DK:DK + 1, :], lnden[DK:DK + 1, :],
                                     mybir.ActivationFunctionType.Exp,
                                     scale=-1.0)
                bcp = mm_ps.tile([128, SB], F32, tag="mm")
                nc.tensor.matmul(bcp[0:DK, :], onesc_sb[64:65, :],
                                 rec16[DK:DK + 1, :],
                                 start=True, stop=True)
                bc = npool.tile([128, SB], MMDT, tag="bc")
                nc.scalar.copy(bc[0:DK, :], bcp[0:DK, :])
                nc.vector.tensor_mul(ot[p][j][DK * u:DK * u + DK, :],
                                     pv[0:DK, :], bc[0:DK, :])

        def wo_phase(jb):
            for st4 in range(4):
                st = 4 * jb + st4
                rq = slice(128 * st4, 128 * st4 + 128)
                for dc in range(2):
                    cols = slice(SB * dc, SB * dc + SB)
                    ps = mm_ps.tile([128, SB], F32, tag="mm")
                    for p in range(2):
                        nc.tensor.matmul(ps[:], ot[p][jb][:, rq], wo_sb[:, p, cols],
                                         start=(p == 0), stop=(p == 1))
                    o_sb = opool.tile([128, SB], F16, tag="osb")
                    out_copy(o_sb[:], ps[:])
                    nc.sync.dma_start(out[128 * st:128 * st + 128, cols], o_sb[:])

        # phase schedule: PE stream stays ~2 phases ahead of its producers
        proj(0)
        proj(1)
        attn(0)
        proj(2)
        wo_phase(0)
        attn(1)
        proj(3)
        wo_phase(1)
        attn(2)
        wo_phase(2)
        attn(3)
        wo_phase(3)

    nc.compile()
    return nc


_CACHED_NC = {}


def _get_program(with_bias=False):
    if with_bias not in _CACHED_NC:
        _CACHED_NC[with_bias] = build_program(with_bias=with_bias)
    return _CACHED_NC[with_bias]


# ---------------------------------------------------------------------------
# entry point
# ---------------------------------------------------------------------------

def kernel(x, token_position, Wq, bq, Wk, bk, Wv, bv, Wo, bo, _results=None):
    from concourse.bass_utils import run_bass_kernel_spmd

    in_maps = make_core_inputs(x, token_position, Wq, bq, Wk, bk, Wv, bv, Wo, bo)
    if _results is None:
        with_bias = any(float(np.abs(np.asarray(v)).max()) != 0.0
                        for v in (bq, bk, bv))
        nc = _get_program(with_bias=with_bias)
        res = run_bass_kernel_spmd(nc, in_maps, list(range(N_CORES)))
        _results = [res.results[i]["out"] for i in range(N_CORES)]
    bo = np.asarray(bo, dtype=np.float32)
    out = np.empty((B, S, D), dtype=np.float32)
    for b in range(B):
        acc = np.asarray(_results[HG * b], dtype=np.float32)
        for hg in range(1, HG):
            acc = acc + np.asarray(_results[HG * b + hg], dtype=np.float32)
        out[b] = acc + bo[None, :]
    return out


# revision 27
# speedup vs baseline: 1.3281x; 1.1455x over previous
"""Trainium2 Bass kernel: causal multi-head attention with interleaved RoPE.

Problem shapes (hardcoded): x [2, 2048, 1024], 16 heads of dk=64.
Sharding: 8 cores = 2 batches x 4 head-groups (4 heads each). Each core
computes its head-slice Q/K/V projections, RoPE, causal attention, and a
partial output through its Wo row-slice; the host sums the 4 partials per
batch and adds bo.

All matmul operands are fp16 (1 col/cycle at the full 2.4GHz PE clock;
fp32r is SBUF-bandwidth limited to ~1.3GHz effective). PSUM accumulation
stays fp32. End-to-end rel err ~6e-4 (budget 2e-2).

RoPE trick: attention scores are invariant to any permutation of the dk
axis applied to both Q and K, so the Wq/Wk columns are permuted on the host
into a "quadrant half-split" layout where each rotation pair partner sits
exactly 16 partitions away inside the same 32-partition quadrant. The DVE
stream_shuffle (a per-quadrant 32-way permute) then produces the swapped
operand, and RoPE becomes: rot = q * cosT + shuffle(q) * sinT with
host-precomputed tables (sinT carries the sign).

Phase order per PE queue: proj0, proj1, attn0, proj2, wo0, attn1, proj3,
wo1, attn2, wo2, attn3, wo3 — keeps the PE stream dependency-slack ahead
of the DVE/ACT producers (rope, exp, normalize) feeding it.
"""

import os
from contextlib import ExitStack

import numpy as np

import concourse.bass as bass
import concourse.mybir as mybir
import concourse.tile as tile

B, S, D, H = 2, 2048, 1024, 16
DK = D // H  # 64
HG = 4  # heads per core
NCOLS = HG * DK  # 256 columns of the projection per core
THETA = 10000.0
SCALE = 1.0 / float(np.sqrt(DK))
N_CORES = 8

F32 = mybir.dt.float32
F32R = mybir.dt.float32r
F16 = mybir.dt.float16
MMDT = F16


def to_f16(a):
    return np.ascontiguousarray(np.asarray(a, dtype=np.float32).astype(np.float16))


# ---------------------------------------------------------------------------
# host-side prep
# ---------------------------------------------------------------------------

def _rope_perm():
    """Within-head column permutation pi: new row r -> original dk index."""
    perm = np.empty(DK, dtype=np.int64)
    for r in range(DK):
        q, m = divmod(r, 32)
        if m < 16:
            perm[r] = 2 * (16 * q + m)
        else:
            perm[r] = 2 * (16 * q + m - 16) + 1
    return perm


_PERM = _rope_perm()
SHUF_MASK = list(range(16, 32)) + list(range(16))  # swap 16-halves per quadrant


def _rope_tables(pos):
    """cosT/sinT [128, S] fp32 for the permuted layout. pos: [S] int."""
    inv_freq = (np.float32(THETA) ** (-(np.arange(0, DK, 2, dtype=np.float32) / np.float32(DK))))  # [32]
    ang = pos.astype(np.float32)[:, None] * inv_freq[None, :]  # [S, 32]
    cos = np.cos(ang)  # [S, 32]
    sin = np.sin(ang)
    cosT = np.empty((128, S), dtype=np.float32)
    sinT = np.empty((128, S), dtype=np.float32)
    for p in range(128):
        r = p % DK
        q, m = divmod(r, 32)
        if m < 16:
            i = 16 * q + m
            sgn = -1.0
        else:
            i = 16 * q + m - 16
            sgn = 1.0
        cosT[p] = cos[:, i]
        sinT[p] = np.float32(sgn) * sin[:, i]
    return cosT, sinT


def make_core_inputs(x, token_position, Wq, bq, Wk, bk, Wv, bv, Wo, bo):
    """Build the 8 per-core input maps."""
    x = np.asarray(x, dtype=np.float32)
    token_position = np.asarray(token_position)
    Wq, Wk, Wv, Wo = (np.asarray(w, dtype=np.float32) for w in (Wq, Wk, Wv, Wo))
    bq, bk, bv = (np.asarray(b_, dtype=np.float32) for b_ in (bq, bk, bv))

    in_maps = []
    tables = {}
    for c in range(N_CORES):
        b, hg = divmod(c, HG)
        heads = range(HG * hg, HG * hg + HG)
        # permuted q/k column indices for this core's heads
        cols_qk = np.concatenate([DK * h + _PERM for h in heads])
        cols_v = np.arange(NCOLS * hg, NCOLS * hg + NCOLS)
        if b not in tables:
            tables[b] = _rope_tables(np.asarray(token_position[b]))
        cosT, sinT = tables[b]
        wo_rows = Wo[cols_v, :]  # [256, 1024] (head-major rows)
        # pair-packed Wo: lane l of pair p holds Wo row of head 2p + l//64,
        # dk l%64 -- so one K=128 matmul contracts a full head pair.
        wo_packed = np.empty((128, 2, D), dtype=np.float32)
        for l_ in range(128):
            for p in range(2):
                h = 2 * p + l_ // 64
                wo_packed[l_, p, :] = wo_rows[DK * h + (l_ % 64), :]
        in_maps.append({
            "xT": to_f16(x[b].T),                               # [1024, 2048]
            "wq": to_f16(Wq[:, cols_qk]),                       # [1024, 256]
            "wk": to_f16(Wk[:, cols_qk]),
            "wv": to_f16(Wv[:, cols_v]),
            "wo": to_f16(wo_packed),                            # [128, 2, 1024]
            "bq": to_f16(bq[cols_qk][None, :]),                 # [1, 256]
            "bk": to_f16(bk[cols_qk][None, :]),
            "bv": to_f16(bv[cols_v][None, :]),
            "ones_row": to_f16(np.ones((1, 512), np.float32)),
            "onesc": to_f16(np.ones((128, DK), np.float32)),
            "cosT": cosT,
            "sinT": sinT,
        })
    return in_maps


# ---------------------------------------------------------------------------
# device program
# ---------------------------------------------------------------------------

def build_program(with_bias=False):
    from concourse import bacc, library_config
    nc = bacc.Bacc("TRN2", debug=False)

    xT = nc.declare_dram_parameter("xT", [D, S], MMDT, isOutput=False).ap()
    wq = nc.declare_dram_parameter("wq", [D, NCOLS], MMDT, isOutput=False).ap()
    wk = nc.declare_dram_parameter("wk", [D, NCOLS], MMDT, isOutput=False).ap()
    wv = nc.declare_dram_parameter("wv", [D, NCOLS], MMDT, isOutput=False).ap()
    wo = nc.declare_dram_parameter("wo", [128, 2, D], MMDT, isOutput=False).ap()
    bq = nc.declare_dram_parameter("bq", [1, NCOLS], MMDT, isOutput=False).ap()
    bk = nc.declare_dram_parameter("bk", [1, NCOLS], MMDT, isOutput=False).ap()
    bv = nc.declare_dram_parameter("bv", [1, NCOLS], MMDT, isOutput=False).ap()
    ones_row_d = nc.declare_dram_parameter("ones_row", [1, 512], MMDT, isOutput=False).ap()
    onesc_d = nc.declare_dram_parameter("onesc", [128, DK], MMDT, isOutput=False).ap()
    cosT = nc.declare_dram_parameter("cosT", [128, S], F32, isOutput=False).ap()
    sinT = nc.declare_dram_parameter("sinT", [128, S], F32, isOutput=False).ap()
    out = nc.declare_dram_parameter("out", [S, D], F16, isOutput=True).ap()

    SB = 512            # sq block width
    NSB = S // SB       # 4
    NST = S // 128      # 16 key tiles / V tiles
    NDC = D // 128      # 8 contraction chunks
    GW = 2              # key tiles per score-psum group
    AUGW = DK + 8       # V head stride (72): 16B-aligned fp16 lhsT starts

    with tile.TileContext(nc) as tc, ExitStack() as ctx:
        nc.gpsimd.load_library(library_config.proxy)
        const = ctx.enter_context(tc.tile_pool(name="const", bufs=1))
        sbig = ctx.enter_context(tc.tile_pool(name="sbig", bufs=1))
        xts = ctx.enter_context(tc.tile_pool(name="xts", bufs=8))
        rtmp = ctx.enter_context(tc.tile_pool(name="rtmp", bufs=2))
        epool = ctx.enter_context(tc.tile_pool(name="epool", bufs=3))
        npool = ctx.enter_context(tc.tile_pool(name="npool", bufs=3))
        opool = ctx.enter_context(tc.tile_pool(name="opool", bufs=4))
        mm_ps = ctx.enter_context(tc.tile_pool(name="mm_ps", bufs=2, space="PSUM"))
        sc_ps = ctx.enter_context(tc.tile_pool(name="sc_ps", bufs=2, space="PSUM"))
        pv_ps = ctx.enter_context(tc.tile_pool(name="pv_ps", bufs=2, space="PSUM"))

        # --- constants / weights resident in SBUF (per-dc tiles: finer deps,
        # so the first projection matmuls start early)
        wq_sb = [const.tile([128, NCOLS], MMDT, tag=f"wq{dc}", name=f"wq{dc}")
                 for dc in range(NDC)]
        wk_sb = [const.tile([128, NCOLS], MMDT, tag=f"wk{dc}", name=f"wk{dc}")
                 for dc in range(NDC)]
        wv_sb = [const.tile([128, NCOLS], MMDT, tag=f"wv{dc}", name=f"wv{dc}")
                 for dc in range(NDC)]
        for dc in range(NDC):
            nc.sync.dma_start(wq_sb[dc][:], wq[128 * dc:128 * dc + 128, :])
            nc.sync.dma_start(wk_sb[dc][:], wk[128 * dc:128 * dc + 128, :])
        cos_sb = const.tile([128, S], F32, tag="cos")
        sin_sb = const.tile([128, S], F32, tag="sin")
        nc.sync.dma_start(cos_sb[:], cosT)
        nc.sync.dma_start(sin_sb[:], sinT)
        for dc in range(NDC):
            nc.sync.dma_start(wv_sb[dc][:], wv[128 * dc:128 * dc + 128, :])
        wo_sb = const.tile([128, 2, D], MMDT, tag="wo")
        nc.sync.dma_start(wo_sb[:], wo)
        if with_bias:
            bq_sb = const.tile([1, NCOLS], MMDT, tag="bq")
            bk_sb = const.tile([1, NCOLS], MMDT, tag="bk")
            bv_sb = const.tile([1, NCOLS], MMDT, tag="bv")
            nc.sync.dma_start(bq_sb[:], bq)
            nc.sync.dma_start(bk_sb[:], bk)
            nc.sync.dma_start(bv_sb[:], bv)
        ones_row = const.tile([1, SB], MMDT, tag="ones_row")
        nc.sync.dma_start(ones_row[:], ones_row_d)
        onesc_sb = const.tile([128, DK], MMDT, tag="onesc")
        nc.sync.dma_start(onesc_sb[:], onesc_d)

        # Q^T per (chunk, sq-block): chunk c holds heads {2c, 2c+1}
        qt = [[sbig.tile([128, SB], MMDT, tag=f"qt{c}_{sb}", name=f"qt{c}_{sb}")
               for sb in range(NSB)] for c in range(2)]
        # per-head K^T, zero-padded to 128 partitions (head data on its chunk
        # rows, the complementary 64 rows zeroed)
        kth = [[sbig.tile([128, SB], MMDT, tag=f"kh{h}_{sb}", name=f"kh{h}_{sb}")
                for sb in range(NSB)] for h in range(HG)]
        for h in range(HG):
            zrows = slice(DK, 128) if h % 2 == 0 else slice(0, DK)
            for sb in range(NSB):
                nc.vector.tensor_scalar_mul(kth[h][sb][zrows, :],
                                            cos_sb[zrows, 0:SB], 0.0)
        # V augmented with a ones column per head, per key tile.
        vaug = [sbig.tile([128, HG * AUGW], MMDT, tag=f"va{st}", name=f"va{st}")
                for st in range(NST)]
        # normalized O^T per (head-pair, sq-block): lanes 0:64 = even head's
        # dk, 64:128 = odd head's dk -- Wo contracts a dense K=128 per pair.
        ot = [[sbig.tile([128, SB], MMDT, tag=f"ot{p}_{j}", name=f"ot{p}_{j}")
               for j in range(NSB)] for p in range(2)]

        ncopy = [0]

        def out_copy(dst, src):
            k = ncopy[0] % 2
            ncopy[0] += 1
            if k == 0:
                nc.vector.tensor_copy(dst, src)
            else:
                nc.scalar.copy(dst, src)

        def proj(sb):
            ss = slice(SB * sb, SB * sb + SB)
            xt_t = []
            for dc in range(NDC):
                t = xts.tile([128, SB], MMDT, tag="xt")
                nc.sync.dma_start(t[:], xT[128 * dc:128 * dc + 128, ss])
                xt_t.append(t)
            for c in range(2):
                ncol = slice(128 * c, 128 * c + 128)
                for (w_sb, bname) in ((wq_sb, "bq"), (wk_sb, "bk")):
                    ps = mm_ps.tile([128, SB], F32, tag="mm")
                    for dc in range(NDC):
                        nc.tensor.matmul(ps[:], w_sb[dc][:, ncol], xt_t[dc][:],
                                         start=(dc == 0),
                                         stop=(dc == NDC - 1 and not with_bias))
                    if with_bias:
                        b_sb = bq_sb if bname == "bq" else bk_sb
                        nc.tensor.matmul(ps[:], b_sb[0:1, ncol], ones_row[0:1, :],
                                         start=False, stop=True)
                    # rope: dst = ps*cos + shuffle(ps)*sin
                    t_cos = rtmp.tile([128, SB], F32, tag="rc")
                    nc.vector.tensor_mul(t_cos[:], ps[:], cos_sb[:, ss])
                    t_shuf = rtmp.tile([128, SB], F32, tag="rs")
                    nc.vector.stream_shuffle(t_shuf[:], ps[:], SHUF_MASK)
                    t_sin = rtmp.tile([128, SB], F32, tag="rm")
                    nc.gpsimd.tensor_mul(t_sin[:], t_shuf[:], sin_sb[:, ss])
                    if bname == "bq":
                        nc.vector.tensor_add(qt[c][sb][:], t_cos[:], t_sin[:])
                    else:
                        nc.vector.tensor_add(kth[2 * c][sb][0:DK, :],
                                             t_cos[0:DK, :], t_sin[0:DK, :])
                        nc.vector.tensor_add(kth[2 * c + 1][sb][DK:128, :],
                                             t_cos[DK:128, :], t_sin[DK:128, :])
            for st4 in range(SB // 128):
                st = (SB // 128) * sb + st4
                ps = mm_ps.tile([128, SB], F32, tag="mm")
                for dc in range(NDC):
                    nc.tensor.matmul(ps[:, 0:NCOLS],
                                     xt_t[dc][:, 128 * st4:128 * st4 + 128],
                                     wv_sb[dc][:],
                                     start=(dc == 0),
                                     stop=(dc == NDC - 1 and not with_bias))
                if with_bias:
                    nc.tensor.matmul(ps[:, 0:NCOLS], ones_row[0:1, 0:128],
                                     bv_sb[0:1, :], start=False, stop=True)
                va = vaug[st][:].rearrange("p (h e) -> p h e", h=HG)
                nc.vector.tensor_copy(va[:, :, 0:DK],
                                      ps[:, 0:NCOLS].rearrange("p (h k) -> p h k", h=HG))
                nc.vector.tensor_copy(va[:, :, DK], onesc_sb[:, 0:HG])

        def attn(j):
            # S^T layout: psum group = GW key tiles x one sq block; exp on ACT
            # over the causally-valid column ranges only; PV accumulates
            # (V | ones) so row 64 is the softmax denominator.

            for h in range(HG):
                c, half = divmod(h, 2)
                pv = pv_ps.tile([128, SB], F32, tag="pv")
                ngrp = (4 * j + 4) // GW
                for g in range(ngrp):
                    sc = sc_ps.tile([128, GW * SB], F32, tag="sc")
                    # lo[t]: first causally-valid query column for key tile
                    # GW*g + t; scores/exp/PV all skip cols below it.
                    los = [min(max(128 * (GW * g + t - 4 * j), 0), SB)
                           for t in range(GW)]
                    for t in range(GW):
                        i = GW * g + t
                        lo = los[t]
                        if lo >= SB:
                            continue
                        nc.tensor.matmul(
                            sc[:, SB * t + lo:SB * t + SB],
                            kth[h][i // 4][:, 128 * (i % 4):128 * (i % 4) + 128],
                            qt[c][j][:, lo:SB],
                            start=True, stop=True)
                    e = epool.tile([128, GW * SB], MMDT, tag="e")
                    diag = GW * g + GW - 4 * j > 0
                    if not diag:
                        # one big exp call: ACT per-call overhead ~0.2us
                        nc.scalar.activation(e[:], sc[:],
                                             mybir.ActivationFunctionType.Exp,
                                             scale=SCALE)
                    else:
                        for t in range(GW):
                            lo = los[t]
                            if lo > 0:
                                # zero the causally-dead prefix: PV streams
                                # the full e width (uniform psum region)
                                nc.gpsimd.memset(e[:, SB * t:SB * t + lo], 0.0)
                            nc.scalar.activation(
                                e[:, SB * t + lo:SB * t + SB],
                                sc[:, SB * t + lo:SB * t + SB],
                                mybir.ActivationFunctionType.Exp,
                                scale=SCALE)
                            if 128 * (GW * g + t - 4 * j) >= 0:
                                # triangle band: zero e where query < key
                                band = e[:, SB * t + lo:SB * t + lo + 128]
                                nc.gpsimd.affine_select(
                                    out=band.rearrange("p (o f) -> p o f", o=1),
                                    in_=band.rearrange("p (o f) -> p o f", o=1),
                                    compare_op=mybir.AluOpType.is_ge,
                                    fill=0.0, base=0,
                                    pattern=[[-128, 1], [1, 128]],
                                    channel_multiplier=-1)
                    for t in range(GW):
                        i = GW * g + t
                        lhs = vaug[i][:].rearrange("p (h e) -> p h e", h=HG)[:, h, 0:DK + 1]
                        nc.tensor.matmul(
                            pv[0:DK + 1, :], lhs, e[:, SB * t:SB * t + SB],
                            start=(g == 0 and t == 0),
                            stop=(g == ngrp - 1 and t == GW - 1))
                # normalize: DVE reciprocal straight from PSUM (table-free;
                # ACT reciprocal would thrash activation-table loads),
                # PE broadcast, then scale the head's 64 output dims
                p, u = divmod(h, 2)
                rec16 = npool.tile([128, SB], MMDT, tag="rec16")
                with nc.allow_low_precision(reason="denominator recip in fp16"):
                    nc.vector.reciprocal(rec16[DK:DK + 1, :], pv[DK:DK + 1, :])
                bcp = mm_ps.tile([128, SB], F32, tag="mm")
                nc.tensor.matmul(bcp[0:DK, :], onesc_sb[64:65, :],
                                 rec16[DK:DK + 1, :],
                                 start=True, stop=True)
                bc = npool.tile([128, SB], MMDT, tag="bc")
                nc.scalar.copy(bc[0:DK, :], bcp[0:DK, :])
                nc.vector.tensor_mul(ot[p][j][DK * u:DK * u + DK, :],
                                     pv[0:DK, :], bc[0:DK, :])

        def wo_phase(jb):
            for st4 in range(4):
                st = 4 * jb + st4
                rq = slice(128 * st4, 128 * st4 + 128)
                for dc in range(2):
                    cols = slice(SB * dc, SB * dc + SB)
                    ps = mm_ps.tile([128, SB], F32, tag="mm")
                    for p in range(2):
                        nc.tensor.matmul(ps[:], ot[p][jb][:, rq], wo_sb[:, p, cols],
                                         start=(p == 0), stop=(p == 1))
                    o_sb = opool.tile([128, SB], F16, tag="osb")
                    out_copy(o_sb[:], ps[:])
                    nc.sync.dma_start(out[128 * st:128 * st + 128, cols], o_sb[:])

        # phase schedule: PE stream stays ~2 phases ahead of its producers
        proj(0)
        proj(1)
        attn(0)
        proj(2)
        wo_phase(0)
        attn(1)
        proj(3)
        wo_phase(1)
        attn(2)
        wo_phase(2)
        attn(3)
        wo_phase(3)

    nc.compile()
    return nc


_CACHED_NC = {}


def _get_program(with_bias=False):
    if with_bias not in _CACHED_NC:
        _CACHED_NC[with_bias] = build_program(with_bias=with_bias)
    return _CACHED_NC[with_bias]


# ---------------------------------------------------------------------------
# entry point
# ---------------------------------------------------------------------------

def kernel(x, token_position, Wq, bq, Wk, bk, Wv, bv, Wo, bo, _results=None):
    from concourse.bass_utils import run_bass_kernel_spmd

    in_maps = make_core_inputs(x, token_position, Wq, bq, Wk, bk, Wv, bv, Wo, bo)
    if _results is None:
        with_bias = any(float(np.abs(np.asarray(v)).max()) != 0.0
                        for v in (bq, bk, bv))
        nc = _get_program(with_bias=with_bias)
        res = run_bass_kernel_spmd(nc, in_maps, list(range(N_CORES)))
        _results = [res.results[i]["out"] for i in range(N_CORES)]
    bo = np.asarray(bo, dtype=np.float32)
    out = np.empty((B, S, D), dtype=np.float32)
    for b in range(B):
        acc = np.asarray(_results[HG * b], dtype=np.float32)
        for hg in range(1, HG):
            acc = acc + np.asarray(_results[HG * b + hg], dtype=np.float32)
        out[b] = acc + bo[None, :]
    return out
